# revision 11
# baseline (speedup 1.0000x reference)
"""Trainium2 Bass kernel for Physics-Attention over an irregular mesh.

Contract: kernel(**inputs) takes the FULL inputs from setup_inputs() and
returns the FULL [4, 32768, 256] f32 output, distributing across 8 cores
internally (one (batch, half-of-N) shard per core; the slice-token pooling
reductions are exchanged with a pairwise AllGather + local add).

Structure per core (16384 tokens):
  pass 1 (128 single token-tiles, software-pipelined by 2): logits/feature
    matmuls, softmax over slices, pooling into PSUM-resident slice-token
    accumulators; routing weights transposed to swT store via the DMA XBAR
    (one dma_start_transpose per tile) so the PE stays dense and HAM-warm.
  exchange: two staggered AllGathers (tiles 0..63 / 64..127), bf16 payload.
  stage: tiny cross-attention among 64 slice tokens (with PE warm-up
    transposes overlapping the vector prologue).
  pass 2: outT = C^T-stationary matmuls over the stored routing weights,
    bf16 results DMAd out, fixed up on the host.
"""

import sys

sys.path.insert(0, "/opt/trn_rl_repo")

import numpy as np
import ml_dtypes

import concourse.bass as bass
import concourse.mybir as mybir
import concourse.tile as tile
from concourse import bacc, bass_utils
from concourse.bass import ts

F32 = mybir.dt.float32
BF16 = mybir.dt.bfloat16
FP8 = mybir.dt.float8e4
NP_FP8 = ml_dtypes.float8_e4m3
AF = mybir.ActivationFunctionType
ALU = mybir.AluOpType
DR = mybir.MatmulPerfMode.DoubleRow

B, N, DIM = 4, 32768, 256
H, D, G = 8, 64, 64
INNER = H * D  # 512
NCORES = 8
NLOC = N // 2          # 16384 tokens per core
TOK = 128              # tokens per tile
T = NLOC // TOK        # 128 tiles
KCH = DIM // 128       # 2 contraction chunks
EPS_SLICE = 1e-5

# knobs
XBAR_T = True          # transpose sw via DMA XBAR instead of the PE
FP8_FX = False         # x@Wfx in fp8 DoubleRow (xt loaded twice: bf16+fp8)
BF16_OUT = True        # write outT in bf16, cast on host
BF16_CC = True         # exchange pooled stats in bf16
PIPE_D = 2             # software pipeline depth (pooling/transpose lag)
WARM_MM = 14           # PE warm-up transposes after the gather lands

_CACHE = {}


def _build(attn_scale: float, res_scale: float):
    nc = bacc.Bacc("TRN2", target_bir_lowering=False, debug=False,
                   enable_asserts=False, num_devices=NCORES)

    xT_d = nc.dram_tensor("xT", [DIM, NLOC], BF16, kind="ExternalInput").ap()
    AT_d = nc.dram_tensor("AT", [DIM, INNER], BF16, kind="ExternalInput").ap()
    fxdt = FP8 if FP8_FX else BF16
    WfxT_d = nc.dram_tensor("WfxT", [DIM, INNER], fxdt, kind="ExternalInput").ap()
    if FP8_FX:
        xT8_d = nc.dram_tensor("xT8", [DIM, NLOC], FP8, kind="ExternalInput").ap()
    idbf_d = nc.dram_tensor("idbf", [128, 128], BF16, kind="ExternalInput").ap()
    id32_d = nc.dram_tensor("id32", [64, 64], F32, kind="ExternalInput").ap()
    WqT_d = nc.dram_tensor("WqT", [2 * D, D], F32, kind="ExternalInput").ap()
    WkT_d = nc.dram_tensor("WkT", [D, D], F32, kind="ExternalInput").ap()
    WvT_d = nc.dram_tensor("WvT", [D, D], F32, kind="ExternalInput").ap()
    WoT_d = nc.dram_tensor("WoT", [128, H, DIM], BF16, kind="ExternalInput").ap()
    odt = BF16 if BF16_OUT else F32
    outT_d = nc.dram_tensor("outT", [2, 128, NLOC], odt, kind="ExternalOutput").ap()
    warm_d = nc.dram_tensor("warm", [64, 64], F32, kind="ExternalOutput").ap()

    xT_v = xT_d.rearrange("(c p) n -> p c n", p=128)    # [128, 2, NLOC]
    AT_v = AT_d.rearrange("(c p) n -> p c n", p=128)    # [128, 2, 512]
    WfxT_v = WfxT_d.rearrange("(c p) n -> p c n", p=128)
    if FP8_FX:
        xT8_v = xT8_d.rearrange("(c p) n -> p c n", p=128)

    ccdt = BF16 if BF16_CC else F32
    E = D + 1

    with tile.TileContext(nc) as tc:
        with (
            tc.tile_pool(name="consts", bufs=1) as consts,
            tc.tile_pool(name="store", bufs=1) as store,
            tc.tile_pool(name="work", bufs=PIPE_D + 2) as work,
            tc.tile_pool(name="small", bufs=PIPE_D + 2) as small,
            tc.tile_pool(name="obuf", bufs=2) as obuf,
            tc.tile_pool(name="stage", bufs=1) as stg_pool,
            tc.tile_pool(name="pslg", bufs=2, space="PSUM") as pslg,
            tc.tile_pool(name="psfx", bufs=2, space="PSUM") as psfx,
            tc.tile_pool(name="psacc", bufs=1, space="PSUM") as psacc,
            tc.tile_pool(name="psst", bufs=2, space="PSUM") as psst,
            tc.tile_pool(name="dram", bufs=1, space="DRAM") as dram,
        ):
            # ---- resident constants (split across the two DGE queues) ----
            AT_sb = consts.tile([128, KCH, INNER], BF16)
            nc.sync.dma_start(AT_sb, AT_v)
            WfxT_sb = consts.tile([128, KCH, INNER], fxdt)
            nc.scalar.dma_start(WfxT_sb, WfxT_v)
            id32 = consts.tile([64, 64], F32)
            nc.sync.dma_start(id32, id32_d)
            idbf = None
            if not XBAR_T:
                idbf = consts.tile([128, 128], BF16)
                nc.scalar.dma_start(idbf, idbf_d)
            WqT2_sb = consts.tile([128, 64], F32)
            nc.sync.dma_start(WqT2_sb, WqT_d)
            WkT_sb = consts.tile([64, 64], F32)
            nc.scalar.dma_start(WkT_sb, WkT_d)
            WvT_sb = consts.tile([64, 64], F32)
            nc.sync.dma_start(WvT_sb, WvT_d)
            WoT2_sb = consts.tile([128, H, DIM], BF16)
            nc.scalar.dma_start(WoT2_sb, WoT_d)

            # transposed routing weights, tile-major: [hg%128, t, chunk, tok]
            swT_store = store.tile([128, T, 4, TOK], BF16)
            # slice-token accumulator: [p, hp, blk, 0:64]=st_un for head
            # h=2*blk+hp at partitions hp*64+g, col 64 = snorm. dim1 selects
            # a bank per partition-half so each half owns its zero region.
            st_ps = psacc.tile([128, 2, 4, 128], F32, name="st_ps")

            # exchange buffers (DRAM round-trip, proven HW path)
            cc_in = [dram.tile([128, 4 * E], ccdt, name=f"cc_in{i}")
                     for i in range(2)]
            cc_out = [dram.tile([2, 128, 4 * E], ccdt, name=f"cc_out{i}")
                      for i in range(2)]

            # preset the ones column of the rotating fxs buffers
            fxs_bufs = []
            for i in range(PIPE_D + 2):
                fxs = work.tile([128, H, E], BF16, tag="fxs")
                nc.gpsimd.memset(fxs[:, :, D], 1.0)
                fxs_bufs.append(fxs)

            # ================= PASS 1 =================
            # software-pipelined: iteration t runs main/softmax for tile t
            # and pooling/transpose for tile t - PIPE_D, so the PE never
            # waits on the softmax chain.
            swn_hist = {}
            fxs_hist = {}
            stun = [None, None]

            def head_block(t):
                xt = work.tile([128, KCH, TOK], BF16, tag="xt")
                nc.sync.dma_start(xt, xT_v[:, :, t * TOK:(t + 1) * TOK])
                if FP8_FX:
                    xt8 = work.tile([128, KCH, TOK], FP8, tag="xt8")
                    nc.sync.dma_start(
                        xt8, xT8_v[:, :, t * TOK:(t + 1) * TOK])
                lg = pslg.tile([128, INNER], F32, tag="lg")
                for k in range(KCH):
                    nc.tensor.matmul(lg, xt[:, k, :], AT_sb[:, k, :],
                                     start=(k == 0), stop=(k == KCH - 1))
                fx = psfx.tile([128, INNER], F32, tag="fx")
                if FP8_FX:
                    nc.tensor.matmul(fx, xt8, WfxT_sb, perf_mode=DR,
                                     start=True, stop=True)
                else:
                    for k in range(KCH):
                        nc.tensor.matmul(fx, xt[:, k, :], WfxT_sb[:, k, :],
                                         start=(k == 0), stop=(k == KCH - 1))

                usw = work.tile([128, H, G], BF16, tag="usw")
                nc.scalar.activation(usw.rearrange("p h g -> p (h g)"),
                                     lg, AF.Exp)
                den = small.tile([128, H], F32, tag="den")
                nc.vector.reduce_sum(den, usw, axis=mybir.AxisListType.X)
                rden = small.tile([128, H], F32, tag="rden")
                nc.vector.reciprocal(rden, den)
                swn = work.tile([128, H, G], BF16, tag="swn")
                nc.gpsimd.tensor_tensor(
                    swn, usw, rden[:, :, None].to_broadcast([128, H, G]),
                    ALU.mult)
                fxs = fxs_bufs[t % (PIPE_D + 2)]
                nc.scalar.copy(
                    fxs[:, :, 0:D], fx.rearrange("p (h d) -> p h d", d=D))
                swn_hist[t] = swn
                fxs_hist[t] = fxs

            def tail_block(t):
                swn2 = swn_hist.pop(t).rearrange("p h g -> p (h g)")
                fxs = fxs_hist.pop(t)
                first, last = t % 64 == 0, t % 64 == 63
                for blk in range(4):
                    for hp in range(2):
                        lhs = swn2[:, blk * 128 + hp * 64:
                                   blk * 128 + (hp + 1) * 64]
                        nc.tensor.matmul(
                            st_ps[64 * hp:64 * (hp + 1), hp, blk, 0:E],
                            lhs, fxs[:, 2 * blk + hp, :],
                            start=(first and blk == 0),
                            stop=(last and blk == 3))
                if XBAR_T:
                    nc.sync.dma_start_transpose(swT_store[:, t], swn2)
                else:
                    swtp = psst.tile([128, 4, TOK], BF16, tag="tp")
                    for blk in range(4):
                        nc.tensor.transpose(swtp[:, blk, :],
                                            swn2[:, ts(blk, 128)], idbf)
                    nc.vector.tensor_copy(swT_store[:, t], swtp)
                if last:
                    half = t // 64
                    stun[half] = stg_pool.tile([128, 4, E], ccdt,
                                               name=f"stun{half}")
                    nc.vector.tensor_copy(stun[half][0:64],
                                          st_ps[0:64, 0, :, 0:E])
                    nc.scalar.copy(stun[half][64:128],
                                   st_ps[64:128, 1, :, 0:E])
                    nc.sync.dma_start(
                        cc_in[half],
                        stun[half].rearrange("p a b -> p (a b)"))
                    nc.gpsimd.collective_compute(
                        "AllGather", ALU.bypass,
                        replica_groups=[[0, 1], [2, 3], [4, 5], [6, 7]],
                        ins=[cc_in[half].opt()],
                        outs=[cc_out[half].rearrange(
                            "r p e -> (r p) e").opt()],
                    )

            for t in range(T + PIPE_D):
                if t < T:
                    head_block(t)
                if t >= PIPE_D:
                    tail_block(t - PIPE_D)

            # ============ STAGE (slice attention, tiny) ============
            # load reduced halves back with g on partitions:
            # cc_out[i] flat = [hp*64+g, blk, E] -> [g, r, hp, blk, E]
            gth = [None, None]
            for i in range(2):
                gth[i] = stg_pool.tile([64, 2, 2, 4, E], ccdt,
                                       name=f"gth{i}")
                src = cc_out[i].rearrange("r (hp g) (blk e) -> g r hp blk e",
                                          hp=2, e=E)
                (nc.sync if i == 0 else nc.scalar).dma_start(gth[i], src)

            # PE warm-up: junk accumulating matmuls reading the gathered tile
            # keep HAM from idling through the vector prologue below. The
            # accumulator is sunk to a throwaway output to satisfy the BIR
            # verifier (every written location needs a reader).
            gflat = gth[1].rearrange("g r hp blk e -> g (r hp blk e)")
            wtp = psst.tile([64, 64], F32, tag="tp", name="wtp_acc")
            for w in range(WARM_MM):
                sl = (w % 15) * 64
                nc.tensor.matmul(wtp, gflat[:, sl:sl + 64],
                                 gflat[:, sl + 64:sl + 128],
                                 start=(w == 0), stop=(w == WARM_MM - 1))
            warm_sb = stg_pool.tile([64, 64], F32, name="warm_sb")
            nc.vector.tensor_copy(warm_sb, wtp)
            nc.sync.dma_start(warm_d, warm_sb)

            # stg[g, hp, blk, e], heads h = 2*blk + hp
            stg = stg_pool.tile([64, 2, 4, E], F32)
            s0 = stg_pool.tile([64, 2, 4, E], F32, name="s0")
            nc.vector.tensor_add(s0, gth[0][:, 0], gth[0][:, 1])
            nc.gpsimd.tensor_add(stg, s0, gth[1][:, 0])
            nc.vector.tensor_add(stg, stg, gth[1][:, 1])
            stgh = stg.rearrange("g hp blk e -> g (hp blk) e")  # kh = hp*4+blk

            snorm_e = stg_pool.tile([64, H], F32)
            nc.vector.tensor_scalar_add(snorm_e, stgh[:, :, D], EPS_SLICE)
            rs = stg_pool.tile([64, H], F32)
            nc.vector.reciprocal(rs, snorm_e)
            st_sb = stg_pool.tile([64, H, D], F32)
            nc.vector.tensor_tensor(st_sb, stgh[:, :, 0:D],
                                    rs[:, :, None].to_broadcast([64, H, D]),
                                    ALU.mult)
            kv = stg_pool.tile([64, D], F32)
            nc.vector.reduce_sum(kv, st_sb.rearrange("p h d -> p d h"),
                                 axis=mybir.AxisListType.X)

            stT = stg_pool.tile([64, H, D], F32)
            for kh in range(H):
                tp = psst.tile([64, 64], F32, tag="tp")
                nc.tensor.transpose(tp, st_sb[:, kh, :], id32)
                nc.vector.tensor_copy(stT[:, kh, :], tp)
            kvT_p = psst.tile([64, 64], F32, tag="tp")
            nc.tensor.transpose(kvT_p, kv, id32)
            kvT = stg_pool.tile([64, D], F32)
            nc.vector.tensor_copy(kvT, kvT_p)

            q_ps = pslg.tile([64, H, D], F32, tag="lg", name="q_ps")
            for kh in range(H):
                nc.tensor.matmul(q_ps[:, kh, :], stT[:, kh, :], WqT2_sb[0:64],
                                 start=(kh == 0), stop=(kh == H - 1))
            k_ps = psfx.tile([64, D], F32, tag="fx", name="k_ps")
            nc.tensor.matmul(k_ps, kvT, WkT_sb, start=True, stop=True)
            k_sb = stg_pool.tile([64, D], F32)
            nc.vector.tensor_copy(k_sb, k_ps)
            v_ps = psfx.tile([64, D], F32, tag="fx", name="v_ps")
            nc.tensor.matmul(v_ps, kvT, WvT_sb, start=True, stop=True)
            v_sb = stg_pool.tile([64, D], F32)
            nc.vector.tensor_copy(v_sb, v_ps)

            def rnorm(src, nh, tag):
                # 1/||src|| = sqrt(1/sum(src^2)) without Ln/Exp table loads
                sq = stg_pool.tile([64, nh, D], F32, name=f"sq_{tag}")
                nc.scalar.activation(sq, src, AF.Square)
                n2 = stg_pool.tile([64, nh], F32, name=f"n2_{tag}")
                nc.vector.reduce_sum(n2, sq, axis=mybir.AxisListType.X)
                rn2 = stg_pool.tile([64, nh], F32, name=f"rn2_{tag}")
                nc.vector.reciprocal(rn2, n2)
                t1 = stg_pool.tile([64, nh], F32, name=f"t1_{tag}")
                nc.scalar.sqrt(t1, rn2)
                return t1

            rq = rnorm(q_ps, H, "q")
            rk = rnorm(k_sb[:, None, :], 1, "k")

            qn = stg_pool.tile([64, H, D], F32)
            nc.vector.tensor_tensor(qn, q_ps,
                                    rq[:, :, None].to_broadcast([64, H, D]),
                                    ALU.mult)
            kn = stg_pool.tile([64, D], F32)
            nc.vector.tensor_tensor(kn, k_sb,
                                    rk[:, 0:1].to_broadcast([64, D]), ALU.mult)

            qnT = stg_pool.tile([64, H, D], F32)
            for kh in range(H):
                tp = psst.tile([64, 64], F32, tag="tp")
                nc.tensor.transpose(tp, qn[:, kh, :], id32)
                nc.vector.tensor_copy(qnT[:, kh, :], tp)
            knT_p = psst.tile([64, 64], F32, tag="tp")
            nc.tensor.transpose(knT_p, kn, id32)
            knT = stg_pool.tile([64, D], F32)
            nc.vector.tensor_copy(knT, knT_p)

            L_ps = pslg.tile([64, H, G], F32, tag="lg", name="L_ps")
            for kh in range(H):
                nc.tensor.matmul(L_ps[:, kh, :], qnT[:, kh, :], knT,
                                 start=(kh == 0), stop=(kh == H - 1))
            e_sb = stg_pool.tile([64, H, G], F32)
            nc.scalar.activation(e_sb, L_ps, AF.Exp, scale=attn_scale)
            aden = stg_pool.tile([64, H], F32)
            nc.vector.reduce_sum(aden, e_sb, axis=mybir.AxisListType.X)
            ra = stg_pool.tile([64, H], F32)
            nc.vector.reciprocal(ra, aden)

            LT_ps = psfx.tile([64, H, G], F32, tag="fx", name="LT_ps")
            for kh in range(H):
                nc.tensor.matmul(LT_ps[:, kh, :], knT, qnT[:, kh, :],
                                 start=(kh == 0), stop=(kh == H - 1))
            eT_sb = stg_pool.tile([64, H, G], F32)
            nc.scalar.activation(eT_sb, LT_ps, AF.Exp, scale=attn_scale)

            av_ps = pslg.tile([64, H, D], F32, tag="lg", name="av_ps")
            for kh in range(H):
                nc.tensor.matmul(av_ps[:, kh, :], eT_sb[:, kh, :], v_sb,
                                 start=(kh == 0), stop=(kh == H - 1))

            os_sb = stg_pool.tile([64, H, D], F32)
            nc.vector.tensor_tensor(os_sb, av_ps,
                                    ra[:, :, None].to_broadcast([64, H, D]),
                                    ALU.mult)
            rst = stg_pool.tile([64, H, D], F32)
            nc.vector.tensor_scalar_mul(rst, st_sb, res_scale)
            nc.vector.tensor_add(os_sb, os_sb, rst)

            osT = stg_pool.tile([64, H, D], BF16)
            for kh in range(H):
                tp = psst.tile([64, 64], F32, tag="tp")
                nc.tensor.transpose(tp, os_sb[:, kh, :], id32)
                nc.vector.tensor_copy(osT[:, kh, :], tp)

            C_sb = stg_pool.tile([128, 4, DIM], BF16)
            for cc in range(4):
                C_ps = psst.tile([128, DIM], F32, tag="tp")
                for par in range(2):
                    kh = par * 4 + cc
                    h = 2 * cc + par
                    nc.tensor.matmul(C_ps[64 * par:64 * par + 64, :],
                                     osT[:, kh, :], WoT2_sb[0:64, h, :],
                                     start=True, stop=True)
                nc.vector.tensor_copy(C_sb[:, cc, :], C_ps)

            # ================= PASS 2 =================
            # outT[f, tok] accumulated over 4 hg-chunks; C slices stationary.
            # op buffers rotate over 4 PSUM banks (pslg + psfx pools).
            QT = 4                       # token-tiles per group
            GRP = QT * TOK               # 512
            NG = NLOC // GRP             # 32
            ob = None
            for fb in range(2):
                for g in range(NG):
                    pool = (pslg, psfx)[g % 2]
                    op = pool.tile([128, QT, TOK], F32,
                                   tag=("lg", "fx")[g % 2], name="op")
                    for cc in range(4):
                        nc.tensor.matmul(
                            op,
                            C_sb[:, cc, fb * 128:(fb + 1) * 128],
                            swT_store[:, g * QT:(g + 1) * QT, cc, :],
                            start=(cc == 0), stop=(cc == 3))
                    if g % 2 == 0:
                        ob = obuf.tile([128, 2, GRP], odt, tag="ob")
                        nc.vector.tensor_copy(ob[:, 0, :],
                                              op.rearrange("p a b -> p (a b)"))
                    else:
                        nc.scalar.copy(ob[:, 1, :],
                                       op.rearrange("p a b -> p (a b)"))
                        eng = nc.sync if (g // 2) % 2 == 0 else nc.scalar
                        eng.dma_start(
                            outT_d[fb, :, (g - 1) * GRP:(g + 1) * GRP],
                            ob.rearrange("p a b -> p (a b)"))

    nc.finalize()
    return nc


def prepare(x, Wfx, bfx, Wx, bx, Wslice, bslice, temp, Wq, Wk, Wv,
            res_scale, attn_scale, Wout, bout):
    x = np.asarray(x, dtype=np.float32)
    Wfx = np.asarray(Wfx, np.float32); bfx = np.asarray(bfx, np.float32)
    Wx = np.asarray(Wx, np.float32); bx = np.asarray(bx, np.float32)
    Wslice = np.asarray(Wslice, np.float32); bslice = np.asarray(bslice, np.float32)
    temp = np.asarray(temp, np.float32).reshape(H)
    Wq = np.asarray(Wq, np.float32); Wk = np.asarray(Wk, np.float32)
    Wv = np.asarray(Wv, np.float32)
    res_scale_f = float(np.asarray(res_scale, np.float32))
    attn = np.asarray(attn_scale, np.float32).reshape(H)
    Wout = np.asarray(Wout, np.float32); bout = np.asarray(bout, np.float32)

    assert np.all(np.abs(bfx) == 0) and np.all(np.abs(bx) == 0) \
        and np.all(np.abs(bslice) == 0), "nonzero projection biases unsupported"
    assert np.ptp(attn) == 0, "non-uniform attn_scale unsupported"
    attn_f = float(attn[0])

    # folded logits weight: logits[:, h*G+g] = x @ ((Wslice @ Wx_h)/temp_h).T
    A = np.concatenate(
        [(Wslice @ Wx[h * D:(h + 1) * D, :]) / temp[h] for h in range(H)], axis=0)
    AT = np.ascontiguousarray(A.T).astype(ml_dtypes.bfloat16)
    if FP8_FX:
        WfxT = np.ascontiguousarray(Wfx.T).astype(NP_FP8)
    else:
        WfxT = np.ascontiguousarray(Wfx.T).astype(ml_dtypes.bfloat16)
    WoT1 = Wout.T.reshape(H, D, DIM).transpose(1, 0, 2)                # [64, 8, 256]
    WoT = np.ascontiguousarray(
        np.concatenate([WoT1, WoT1], axis=0)).astype(ml_dtypes.bfloat16)
    WqT1 = np.ascontiguousarray(Wq.T)
    WqT = np.concatenate([WqT1, WqT1], axis=0)                         # [128, 64]
    WkT = np.ascontiguousarray(Wk.T) / H
    WvT = np.ascontiguousarray(Wv.T) / H
    idbf = np.eye(128, dtype=np.float32).astype(ml_dtypes.bfloat16)
    id32 = np.eye(64, dtype=np.float32)

    key = (attn_f, res_scale_f)
    if key not in _CACHE:
        _CACHE[key] = _build(attn_f, res_scale_f)
    nc = _CACHE[key]

    in_maps = []
    for c in range(NCORES):
        b, half = c // 2, c % 2
        xs = x[b, half * NLOC:(half + 1) * NLOC, :]       # [16384, 256]
        xT = np.ascontiguousarray(xs.T.astype(ml_dtypes.bfloat16))
        im = dict(xT=xT, AT=AT, WfxT=WfxT, idbf=idbf, id32=id32,
                  WqT=WqT, WkT=WkT, WvT=WvT, WoT=WoT)
        if FP8_FX:
            im["xT8"] = np.ascontiguousarray(xs.T.astype(NP_FP8))
        in_maps.append(im)

    def gather(core_outs):
        out = np.empty((B, N, DIM), np.float32)
        for c in range(NCORES):
            b, half = c // 2, c % 2
            oT = np.asarray(core_outs[c]).reshape(DIM, NLOC)
            out[b, half * NLOC:(half + 1) * NLOC, :] = \
                oT.T.astype(np.float32)
        if np.any(bout):
            out += bout
        return out

    return dict(nc=nc, in_maps=in_maps, gather=gather)


def kernel(**inputs):
    prep = prepare(**inputs)
    global _LAST_IN_MAPS
    _LAST_IN_MAPS = prep["in_maps"]
    res = bass_utils.run_bass_kernel_spmd(
        prep["nc"], prep["in_maps"], core_ids=list(range(NCORES)))
    return prep["gather"]([res.results[c]["outT"] for c in range(NCORES)])


# revision 16
# speedup vs baseline: 1.5410x; 1.5410x over previous
"""Trainium2 Bass kernel for Physics-Attention over an irregular mesh.

Contract: kernel(**inputs) takes the FULL inputs from setup_inputs() and
returns the FULL [4, 32768, 256] f32 output, distributing across 8 cores
internally (one (batch, half-of-N) shard per core; the slice-token pooling
reductions are exchanged with a pairwise AllGather + local add).

Structure per core (16384 tokens):
  pass 1 (128 single token-tiles, software-pipelined by 2): logits/feature
    matmuls, softmax over slices, pooling into PSUM-resident slice-token
    accumulators; routing weights transposed to swT store via the DMA XBAR
    (one dma_start_transpose per tile) so the PE stays dense and HAM-warm.
  exchange: two staggered AllGathers (tiles 0..63 / 64..127), bf16 payload.
  stage: tiny cross-attention among 64 slice tokens (with PE warm-up
    transposes overlapping the vector prologue).
  pass 2: outT = C^T-stationary matmuls over the stored routing weights,
    bf16 results DMAd out, fixed up on the host.
"""

import sys

sys.path.insert(0, "/opt/trn_rl_repo")

import numpy as np
import ml_dtypes

import concourse.bass as bass
import concourse.mybir as mybir
import concourse.tile as tile
from concourse import bacc, bass_utils
from concourse.bass import ts

F32 = mybir.dt.float32
BF16 = mybir.dt.bfloat16
FP8 = mybir.dt.float8e4
NP_FP8 = ml_dtypes.float8_e4m3
AF = mybir.ActivationFunctionType
ALU = mybir.AluOpType
DR = mybir.MatmulPerfMode.DoubleRow

B, N, DIM = 4, 32768, 256
H, D, G = 8, 64, 64
INNER = H * D  # 512
NCORES = 8
NLOC = N // 2          # 16384 tokens per core
TOK = 128              # tokens per tile
T = NLOC // TOK        # 128 tiles
KCH = DIM // 128       # 2 contraction chunks
EPS_SLICE = 1e-5

# knobs
XBAR_T = True          # transpose sw via DMA XBAR instead of the PE
FP8_FX = False         # x@Wfx in fp8 DoubleRow (xt loaded twice: bf16+fp8)
BF16_OUT = True        # write outT in bf16, cast on host
BF16_CC = True         # exchange pooled stats in bf16
PIPE_D = 2             # software pipeline depth (pooling/transpose lag)
WARM_MM = 14           # PE warm-up transposes after the gather lands

_CACHE = {}


def _build(attn_scale: float, res_scale: float):
    nc = bacc.Bacc("TRN2", target_bir_lowering=False, debug=False,
                   enable_asserts=False, num_devices=NCORES)

    xT_d = nc.dram_tensor("xT", [DIM, NLOC], BF16, kind="ExternalInput").ap()
    AT_d = nc.dram_tensor("AT", [DIM, INNER], BF16, kind="ExternalInput").ap()
    fxdt = FP8 if FP8_FX else BF16
    WfxT_d = nc.dram_tensor("WfxT", [DIM, INNER], fxdt, kind="ExternalInput").ap()
    if FP8_FX:
        xT8_d = nc.dram_tensor("xT8", [DIM, NLOC], FP8, kind="ExternalInput").ap()
    idbf_d = nc.dram_tensor("idbf", [128, 128], BF16, kind="ExternalInput").ap()
    id32_d = nc.dram_tensor("id32", [64, 64], F32, kind="ExternalInput").ap()
    WqT_d = nc.dram_tensor("WqT", [2 * D, D], F32, kind="ExternalInput").ap()
    WkT_d = nc.dram_tensor("WkT", [D, D], F32, kind="ExternalInput").ap()
    WvT_d = nc.dram_tensor("WvT", [D, D], F32, kind="ExternalInput").ap()
    WoT_d = nc.dram_tensor("WoT", [128, H, DIM], BF16, kind="ExternalInput").ap()
    odt = BF16 if BF16_OUT else F32
    outT_d = nc.dram_tensor("outT", [2, 128, NLOC], odt, kind="ExternalOutput").ap()
    warm_d = nc.dram_tensor("warm", [64, 64], F32, kind="ExternalOutput").ap()

    xT_v = xT_d.rearrange("(c p) n -> p c n", p=128)    # [128, 2, NLOC]
    AT_v = AT_d.rearrange("(c p) n -> p c n", p=128)    # [128, 2, 512]
    WfxT_v = WfxT_d.rearrange("(c p) n -> p c n", p=128)
    if FP8_FX:
        xT8_v = xT8_d.rearrange("(c p) n -> p c n", p=128)

    ccdt = BF16 if BF16_CC else F32
    E = D + 1

    with tile.TileContext(nc) as tc:
        with (
            tc.tile_pool(name="consts", bufs=1) as consts,
            tc.tile_pool(name="store", bufs=1) as store,
            tc.tile_pool(name="work", bufs=PIPE_D + 2) as work,
            tc.tile_pool(name="small", bufs=PIPE_D + 2) as small,
            tc.tile_pool(name="obuf", bufs=2) as obuf,
            tc.tile_pool(name="stage", bufs=1) as stg_pool,
            tc.tile_pool(name="pslg", bufs=2, space="PSUM") as pslg,
            tc.tile_pool(name="psfx", bufs=2, space="PSUM") as psfx,
            tc.tile_pool(name="psacc", bufs=1, space="PSUM") as psacc,
            tc.tile_pool(name="psst", bufs=2, space="PSUM") as psst,
            tc.tile_pool(name="dram", bufs=1, space="DRAM") as dram,
        ):
            # ---- resident constants (split across the two DGE queues) ----
            AT_sb = consts.tile([128, KCH, INNER], BF16)
            nc.sync.dma_start(AT_sb, AT_v)
            WfxT_sb = consts.tile([128, KCH, INNER], fxdt)
            nc.scalar.dma_start(WfxT_sb, WfxT_v)
            id32 = consts.tile([64, 64], F32)
            nc.sync.dma_start(id32, id32_d)
            idbf = None
            if not XBAR_T:
                idbf = consts.tile([128, 128], BF16)
                nc.scalar.dma_start(idbf, idbf_d)
            WqT2_sb = consts.tile([128, 64], F32)
            nc.sync.dma_start(WqT2_sb, WqT_d)
            WkT_sb = consts.tile([64, 64], F32)
            nc.scalar.dma_start(WkT_sb, WkT_d)
            WvT_sb = consts.tile([64, 64], F32)
            nc.sync.dma_start(WvT_sb, WvT_d)
            WoT2_sb = consts.tile([128, H, DIM], BF16)
            nc.scalar.dma_start(WoT2_sb, WoT_d)

            # transposed routing weights, tile-major: [hg%128, t, chunk, tok]
            swT_store = store.tile([128, T, 4, TOK], BF16)
            # slice-token accumulator: [p, hp, blk, 0:64]=st_un for head
            # h=2*blk+hp at partitions hp*64+g, col 64 = snorm. dim1 selects
            # a bank per partition-half so each half owns its zero region.
            st_ps = psacc.tile([128, 2, 4, 128], F32, name="st_ps")

            # exchange buffers (DRAM round-trip, proven HW path)
            cc_in = [dram.tile([128, 4 * E], ccdt, name=f"cc_in{i}")
                     for i in range(2)]
            cc_out = [dram.tile([2, 128, 4 * E], ccdt, name=f"cc_out{i}")
                      for i in range(2)]

            # preset the ones column of the rotating fxs buffers
            fxs_bufs = []
            for i in range(PIPE_D + 2):
                fxs = work.tile([128, H, E], BF16, tag="fxs")
                nc.gpsimd.memset(fxs[:, :, D], 1.0)
                fxs_bufs.append(fxs)

            # ================= PASS 1 =================
            # software-pipelined: iteration t runs main/softmax for tile t
            # and pooling/transpose for tile t - PIPE_D, so the PE never
            # waits on the softmax chain. xt loads, swn buffers and XBAR
            # transposes are paired over two tiles to halve DGE-queue
            # instruction overhead.
            swn_hist = {}
            fxs_hist = {}
            xt_hist = {}
            stun = [None, None]

            def head_block(t):
                if t % 2 == 0:
                    xt2 = work.tile([128, KCH, 2 * TOK], BF16, tag="xt",
                                    name=f"xt{t}")
                    nc.scalar.dma_start(
                        xt2, xT_v[:, :, t * TOK:(t + 2) * TOK])
                    xt_hist[t] = xt_hist[t + 1] = xt2
                    if FP8_FX:
                        xt28 = work.tile([128, KCH, 2 * TOK], FP8, tag="xt8",
                                         name=f"xt8_{t}")
                        nc.scalar.dma_start(
                            xt28, xT8_v[:, :, t * TOK:(t + 2) * TOK])
                        xt_hist[(t, 8)] = xt_hist[(t + 1, 8)] = xt28
                o = (t % 2) * TOK
                xt = xt_hist.pop(t)
                lg = pslg.tile([128, INNER], F32, tag="lg")
                for k in range(KCH):
                    nc.tensor.matmul(lg, xt[:, k, o:o + TOK], AT_sb[:, k, :],
                                     start=(k == 0), stop=(k == KCH - 1))
                fx = psfx.tile([128, INNER], F32, tag="fx")
                if FP8_FX:
                    xt8 = xt_hist.pop((t, 8))
                    nc.tensor.matmul(fx, xt8[:, :, o:o + TOK], WfxT_sb,
                                     perf_mode=DR, start=True, stop=True)
                else:
                    for k in range(KCH):
                        nc.tensor.matmul(fx, xt[:, k, o:o + TOK],
                                         WfxT_sb[:, k, :],
                                         start=(k == 0), stop=(k == KCH - 1))

                usw = work.tile([128, H, G], BF16, tag="usw")
                nc.scalar.activation(usw.rearrange("p h g -> p (h g)"),
                                     lg, AF.Exp)
                den = small.tile([128, H], F32, tag="den")
                nc.vector.reduce_sum(den, usw, axis=mybir.AxisListType.X)
                rden = small.tile([128, H], F32, tag="rden")
                nc.vector.reciprocal(rden, den)
                if t % 2 == 0:
                    swn_hist[t // 2] = work.tile([128, 2, INNER], BF16,
                                                 tag="swnp",
                                                 name=f"swnp{t // 2}")
                swnp = swn_hist[t // 2]
                nc.gpsimd.tensor_tensor(
                    swnp[:, t % 2].rearrange("p (h g) -> p h g", g=G), usw,
                    rden[:, :, None].to_broadcast([128, H, G]),
                    ALU.mult)
                fxs = fxs_bufs[t % (PIPE_D + 2)]
                nc.vector.tensor_copy(
                    fxs[:, :, 0:D], fx.rearrange("p (h d) -> p h d", d=D))
                fxs_hist[t] = fxs

            def tail_block(t):
                swnp = swn_hist[t // 2]
                swn2 = swnp[:, t % 2]
                fxs = fxs_hist.pop(t)
                first, last = t % 64 == 0, t % 64 == 63
                for blk in range(4):
                    for hp in range(2):
                        lhs = swn2[:, blk * 128 + hp * 64:
                                   blk * 128 + (hp + 1) * 64]
                        nc.tensor.matmul(
                            st_ps[64 * hp:64 * (hp + 1), hp, blk, 0:E],
                            lhs, fxs[:, 2 * blk + hp, :],
                            start=(first and blk == 0),
                            stop=(last and blk == 3))
                if XBAR_T:
                    if t % 2 == 1:
                        # one XBAR transpose covers both tiles of the pair:
                        # in [128, 1024] -> out [128, 8, 128] which is
                        # exactly swT_store[:, t-1:t+1] flattened.
                        nc.sync.dma_start_transpose(
                            swT_store[:, t - 1:t + 1].rearrange(
                                "p a b c -> p (a b) c"),
                            swnp.rearrange("p a b -> p (a b)"))
                        del swn_hist[t // 2]
                else:
                    swtp = psst.tile([128, 4, TOK], BF16, tag="tp")
                    for blk in range(4):
                        nc.tensor.transpose(swtp[:, blk, :],
                                            swn2[:, ts(blk, 128)], idbf)
                    nc.vector.tensor_copy(swT_store[:, t], swtp)
                    if t % 2 == 1:
                        del swn_hist[t // 2]
                if last:
                    half = t // 64
                    stun[half] = stg_pool.tile([128, 4, E], ccdt,
                                               name=f"stun{half}")
                    nc.vector.tensor_copy(stun[half][0:64],
                                          st_ps[0:64, 0, :, 0:E])
                    nc.scalar.copy(stun[half][64:128],
                                   st_ps[64:128, 1, :, 0:E])
                    nc.sync.dma_start(
                        cc_in[half],
                        stun[half].rearrange("p a b -> p (a b)"))
                    nc.gpsimd.collective_compute(
                        "AllGather", ALU.bypass,
                        replica_groups=[[0, 1], [2, 3], [4, 5], [6, 7]],
                        ins=[cc_in[half].opt()],
                        outs=[cc_out[half].rearrange(
                            "r p e -> (r p) e").opt()],
                    )

            for t in range(T + PIPE_D):
                if t < T:
                    head_block(t)
                if t >= PIPE_D:
                    tail_block(t - PIPE_D)

            # ============ STAGE (slice attention, tiny) ============
            # load reduced halves back with g on partitions:
            # cc_out[i] flat = [hp*64+g, blk, E] -> [g, r, hp, blk, E]
            gth = [None, None]
            for i in range(2):
                gth[i] = stg_pool.tile([64, 2, 2, 4, E], ccdt,
                                       name=f"gth{i}")
                src = cc_out[i].rearrange("r (hp g) (blk e) -> g r hp blk e",
                                          hp=2, e=E)
                (nc.sync if i == 0 else nc.scalar).dma_start(gth[i], src)

            # PE warm-up: junk accumulating matmuls reading the gathered tile
            # keep HAM from idling through the vector prologue below. The
            # accumulator is sunk to a throwaway output to satisfy the BIR
            # verifier (every written location needs a reader).
            gflat = gth[1].rearrange("g r hp blk e -> g (r hp blk e)")
            wtp = psst.tile([64, 64], F32, tag="tp", name="wtp_acc")
            for w in range(WARM_MM):
                sl = (w % 15) * 64
                nc.tensor.matmul(wtp, gflat[:, sl:sl + 64],
                                 gflat[:, sl + 64:sl + 128],
                                 start=(w == 0), stop=(w == WARM_MM - 1))
            warm_sb = stg_pool.tile([64, 64], F32, name="warm_sb")
            nc.vector.tensor_copy(warm_sb, wtp)
            nc.sync.dma_start(warm_d, warm_sb)

            # stg[g, hp, blk, e], heads h = 2*blk + hp
            stg = stg_pool.tile([64, 2, 4, E], F32)
            s0 = stg_pool.tile([64, 2, 4, E], F32, name="s0")
            nc.vector.tensor_add(s0, gth[0][:, 0], gth[0][:, 1])
            nc.gpsimd.tensor_add(stg, s0, gth[1][:, 0])
            nc.vector.tensor_add(stg, stg, gth[1][:, 1])
            stgh = stg.rearrange("g hp blk e -> g (hp blk) e")  # kh = hp*4+blk

            snorm_e = stg_pool.tile([64, H], F32)
            nc.vector.tensor_scalar_add(snorm_e, stgh[:, :, D], EPS_SLICE)
            rs = stg_pool.tile([64, H], F32)
            nc.vector.reciprocal(rs, snorm_e)
            st_sb = stg_pool.tile([64, H, D], F32)
            nc.vector.tensor_tensor(st_sb, stgh[:, :, 0:D],
                                    rs[:, :, None].to_broadcast([64, H, D]),
                                    ALU.mult)
            kv = stg_pool.tile([64, D], F32)
            nc.vector.reduce_sum(kv, st_sb.rearrange("p h d -> p d h"),
                                 axis=mybir.AxisListType.X)

            stT = stg_pool.tile([64, H, D], F32)
            for kh in range(H):
                tp = psst.tile([64, 64], F32, tag="tp")
                nc.tensor.transpose(tp, st_sb[:, kh, :], id32)
                nc.vector.tensor_copy(stT[:, kh, :], tp)
            kvT_p = psst.tile([64, 64], F32, tag="tp")
            nc.tensor.transpose(kvT_p, kv, id32)
            kvT = stg_pool.tile([64, D], F32)
            nc.vector.tensor_copy(kvT, kvT_p)

            q_ps = pslg.tile([64, H, D], F32, tag="lg", name="q_ps")
            for kh in range(H):
                nc.tensor.matmul(q_ps[:, kh, :], stT[:, kh, :], WqT2_sb[0:64],
                                 start=(kh == 0), stop=(kh == H - 1))
            k_ps = psfx.tile([64, D], F32, tag="fx", name="k_ps")
            nc.tensor.matmul(k_ps, kvT, WkT_sb, start=True, stop=True)
            k_sb = stg_pool.tile([64, D], F32)
            nc.vector.tensor_copy(k_sb, k_ps)
            v_ps = psfx.tile([64, D], F32, tag="fx", name="v_ps")
            nc.tensor.matmul(v_ps, kvT, WvT_sb, start=True, stop=True)
            v_sb = stg_pool.tile([64, D], F32)
            nc.vector.tensor_copy(v_sb, v_ps)

            def rnorm(src, nh, tag):
                # 1/||src|| = sqrt(1/sum(src^2)) without Ln/Exp table loads
                sq = stg_pool.tile([64, nh, D], F32, name=f"sq_{tag}")
                nc.scalar.activation(sq, src, AF.Square)
                n2 = stg_pool.tile([64, nh], F32, name=f"n2_{tag}")
                nc.vector.reduce_sum(n2, sq, axis=mybir.AxisListType.X)
                rn2 = stg_pool.tile([64, nh], F32, name=f"rn2_{tag}")
                nc.vector.reciprocal(rn2, n2)
                t1 = stg_pool.tile([64, nh], F32, name=f"t1_{tag}")
                nc.scalar.sqrt(t1, rn2)
                return t1

            rq = rnorm(q_ps, H, "q")
            rk = rnorm(k_sb[:, None, :], 1, "k")

            qn = stg_pool.tile([64, H, D], F32)
            nc.vector.tensor_tensor(qn, q_ps,
                                    rq[:, :, None].to_broadcast([64, H, D]),
                                    ALU.mult)
            kn = stg_pool.tile([64, D], F32)
            nc.vector.tensor_tensor(kn, k_sb,
                                    rk[:, 0:1].to_broadcast([64, D]), ALU.mult)

            qnT = stg_pool.tile([64, H, D], F32)
            for kh in range(H):
                tp = psst.tile([64, 64], F32, tag="tp")
                nc.tensor.transpose(tp, qn[:, kh, :], id32)
                nc.vector.tensor_copy(qnT[:, kh, :], tp)
            knT_p = psst.tile([64, 64], F32, tag="tp")
            nc.tensor.transpose(knT_p, kn, id32)
            knT = stg_pool.tile([64, D], F32)
            nc.vector.tensor_copy(knT, knT_p)

            L_ps = pslg.tile([64, H, G], F32, tag="lg", name="L_ps")
            for kh in range(H):
                nc.tensor.matmul(L_ps[:, kh, :], qnT[:, kh, :], knT,
                                 start=(kh == 0), stop=(kh == H - 1))
            e_sb = stg_pool.tile([64, H, G], F32)
            nc.scalar.activation(e_sb, L_ps, AF.Exp, scale=attn_scale)
            aden = stg_pool.tile([64, H], F32)
            nc.vector.reduce_sum(aden, e_sb, axis=mybir.AxisListType.X)
            ra = stg_pool.tile([64, H], F32)
            nc.vector.reciprocal(ra, aden)

            LT_ps = psfx.tile([64, H, G], F32, tag="fx", name="LT_ps")
            for kh in range(H):
                nc.tensor.matmul(LT_ps[:, kh, :], knT, qnT[:, kh, :],
                                 start=(kh == 0), stop=(kh == H - 1))
            eT_sb = stg_pool.tile([64, H, G], F32)
            nc.scalar.activation(eT_sb, LT_ps, AF.Exp, scale=attn_scale)

            av_ps = pslg.tile([64, H, D], F32, tag="lg", name="av_ps")
            for kh in range(H):
                nc.tensor.matmul(av_ps[:, kh, :], eT_sb[:, kh, :], v_sb,
                                 start=(kh == 0), stop=(kh == H - 1))

            os_sb = stg_pool.tile([64, H, D], F32)
            nc.vector.tensor_tensor(os_sb, av_ps,
                                    ra[:, :, None].to_broadcast([64, H, D]),
                                    ALU.mult)
            rst = stg_pool.tile([64, H, D], F32)
            nc.vector.tensor_scalar_mul(rst, st_sb, res_scale)
            nc.vector.tensor_add(os_sb, os_sb, rst)

            osT = stg_pool.tile([64, H, D], BF16)
            for kh in range(H):
                tp = psst.tile([64, 64], F32, tag="tp")
                nc.tensor.transpose(tp, os_sb[:, kh, :], id32)
                nc.vector.tensor_copy(osT[:, kh, :], tp)

            C_sb = stg_pool.tile([128, 4, DIM], BF16)
            for cc in range(4):
                C_ps = psst.tile([128, DIM], F32, tag="tp")
                for par in range(2):
                    kh = par * 4 + cc
                    h = 2 * cc + par
                    nc.tensor.matmul(C_ps[64 * par:64 * par + 64, :],
                                     osT[:, kh, :], WoT2_sb[0:64, h, :],
                                     start=True, stop=True)
                nc.vector.tensor_copy(C_sb[:, cc, :], C_ps)

            # ================= PASS 2 =================
            # outT[f, tok] accumulated over 4 hg-chunks; C slices stationary.
            # op buffers rotate over 4 PSUM banks (pslg + psfx pools).
            QT = 4                       # token-tiles per group
            GRP = QT * TOK               # 512
            NG = NLOC // GRP             # 32
            ob = None
            for fb in range(2):
                for g in range(NG):
                    pool = (pslg, psfx)[g % 2]
                    op = pool.tile([128, QT, TOK], F32,
                                   tag=("lg", "fx")[g % 2], name="op")
                    for cc in range(4):
                        nc.tensor.matmul(
                            op,
                            C_sb[:, cc, fb * 128:(fb + 1) * 128],
                            swT_store[:, g * QT:(g + 1) * QT, cc, :],
                            start=(cc == 0), stop=(cc == 3))
                    if g % 2 == 0:
                        ob = obuf.tile([128, 2, GRP], odt, tag="ob")
                        nc.vector.tensor_copy(ob[:, 0, :],
                                              op.rearrange("p a b -> p (a b)"))
                    else:
                        nc.scalar.copy(ob[:, 1, :],
                                       op.rearrange("p a b -> p (a b)"))
                        eng = nc.sync if (g // 2) % 2 == 0 else nc.scalar
                        eng.dma_start(
                            outT_d[fb, :, (g - 1) * GRP:(g + 1) * GRP],
                            ob.rearrange("p a b -> p (a b)"))

    nc.finalize()
    return nc


def prepare(x, Wfx, bfx, Wx, bx, Wslice, bslice, temp, Wq, Wk, Wv,
            res_scale, attn_scale, Wout, bout):
    x = np.asarray(x, dtype=np.float32)
    Wfx = np.asarray(Wfx, np.float32); bfx = np.asarray(bfx, np.float32)
    Wx = np.asarray(Wx, np.float32); bx = np.asarray(bx, np.float32)
    Wslice = np.asarray(Wslice, np.float32); bslice = np.asarray(bslice, np.float32)
    temp = np.asarray(temp, np.float32).reshape(H)
    Wq = np.asarray(Wq, np.float32); Wk = np.asarray(Wk, np.float32)
    Wv = np.asarray(Wv, np.float32)
    res_scale_f = float(np.asarray(res_scale, np.float32))
    attn = np.asarray(attn_scale, np.float32).reshape(H)
    Wout = np.asarray(Wout, np.float32); bout = np.asarray(bout, np.float32)

    assert np.all(np.abs(bfx) == 0) and np.all(np.abs(bx) == 0) \
        and np.all(np.abs(bslice) == 0), "nonzero projection biases unsupported"
    assert np.ptp(attn) == 0, "non-uniform attn_scale unsupported"
    attn_f = float(attn[0])

    # folded logits weight: logits[:, h*G+g] = x @ ((Wslice @ Wx_h)/temp_h).T
    A = np.concatenate(
        [(Wslice @ Wx[h * D:(h + 1) * D, :]) / temp[h] for h in range(H)], axis=0)
    AT = np.ascontiguousarray(A.T).astype(ml_dtypes.bfloat16)
    if FP8_FX:
        WfxT = np.ascontiguousarray(Wfx.T).astype(NP_FP8)
    else:
        WfxT = np.ascontiguousarray(Wfx.T).astype(ml_dtypes.bfloat16)
    WoT1 = Wout.T.reshape(H, D, DIM).transpose(1, 0, 2)                # [64, 8, 256]
    WoT = np.ascontiguousarray(
        np.concatenate([WoT1, WoT1], axis=0)).astype(ml_dtypes.bfloat16)
    WqT1 = np.ascontiguousarray(Wq.T)
    WqT = np.concatenate([WqT1, WqT1], axis=0)                         # [128, 64]
    WkT = np.ascontiguousarray(Wk.T) / H
    WvT = np.ascontiguousarray(Wv.T) / H
    idbf = np.eye(128, dtype=np.float32).astype(ml_dtypes.bfloat16)
    id32 = np.eye(64, dtype=np.float32)

    key = (attn_f, res_scale_f)
    if key not in _CACHE:
        _CACHE[key] = _build(attn_f, res_scale_f)
    nc = _CACHE[key]

    in_maps = []
    for c in range(NCORES):
        b, half = c // 2, c % 2
        xs = x[b, half * NLOC:(half + 1) * NLOC, :]       # [16384, 256]
        xT = np.ascontiguousarray(xs.T.astype(ml_dtypes.bfloat16))
        im = dict(xT=xT, AT=AT, WfxT=WfxT, idbf=idbf, id32=id32,
                  WqT=WqT, WkT=WkT, WvT=WvT, WoT=WoT)
        if FP8_FX:
            im["xT8"] = np.ascontiguousarray(xs.T.astype(NP_FP8))
        in_maps.append(im)

    def gather(core_outs):
        out = np.empty((B, N, DIM), np.float32)
        for c in range(NCORES):
            b, half = c // 2, c % 2
            oT = np.asarray(core_outs[c]).reshape(DIM, NLOC)
            out[b, half * NLOC:(half + 1) * NLOC, :] = \
                oT.T.astype(np.float32)
        if np.any(bout):
            out += bout
        return out

    return dict(nc=nc, in_maps=in_maps, gather=gather)


def kernel(**inputs):
    prep = prepare(**inputs)
    global _LAST_IN_MAPS
    _LAST_IN_MAPS = prep["in_maps"]
    res = bass_utils.run_bass_kernel_spmd(
        prep["nc"], prep["in_maps"], core_ids=list(range(NCORES)))
    return prep["gather"]([res.results[c]["outT"] for c in range(NCORES)])


# revision 25
# speedup vs baseline: 1.6359x; 1.0616x over previous
"""Trainium2 Bass kernel for Physics-Attention over an irregular mesh.

Contract: kernel(**inputs) takes the FULL inputs from setup_inputs() and
returns the FULL [4, 32768, 256] f32 output, distributing across 8 cores
internally (one (batch, half-of-N) shard per core; the slice-token pooling
reductions are exchanged with a pairwise AllGather + local add).

Structure per core (16384 tokens):
  pass 1 (128 single token-tiles, software-pipelined by 2): logits/feature
    matmuls, softmax over slices, pooling into PSUM-resident slice-token
    accumulators; routing weights transposed to swT store via the DMA XBAR
    (one dma_start_transpose per tile) so the PE stays dense and HAM-warm.
  exchange: two staggered AllGathers (tiles 0..63 / 64..127), bf16 payload.
  stage: tiny cross-attention among 64 slice tokens (with PE warm-up
    transposes overlapping the vector prologue).
  pass 2: outT = C^T-stationary matmuls over the stored routing weights,
    bf16 results DMAd out, fixed up on the host.
"""

import sys

sys.path.insert(0, "/opt/trn_rl_repo")

import numpy as np
import ml_dtypes

import concourse.bass as bass
import concourse.mybir as mybir
import concourse.tile as tile
from concourse import bacc, bass_utils
from concourse.bass import ts

F32 = mybir.dt.float32
BF16 = mybir.dt.bfloat16
FP8 = mybir.dt.float8e4
NP_FP8 = ml_dtypes.float8_e4m3
AF = mybir.ActivationFunctionType
ALU = mybir.AluOpType
DR = mybir.MatmulPerfMode.DoubleRow

B, N, DIM = 4, 32768, 256
H, D, G = 8, 64, 64
INNER = H * D  # 512
NCORES = 8
NLOC = N // 2          # 16384 tokens per core
TOK = 128              # tokens per tile
T = NLOC // TOK        # 128 tiles
KCH = DIM // 128       # 2 contraction chunks
EPS_SLICE = 1e-5

# knobs
XBAR_T = True          # transpose sw via DMA XBAR instead of the PE
FP8_FX = False         # x@Wfx in fp8 DoubleRow (xt loaded twice: bf16+fp8)
BF16_OUT = True        # write outT in bf16, cast on host
BF16_CC = True         # exchange pooled stats in bf16
PIPE_D = 2             # software pipeline depth (pooling/transpose lag)
PF = 7                 # xt prefetch distance in tile-pairs (rides out the
                       # DGE-queue wedge while a collective is receiving)
WARM_MM = 14           # PE warm-up matmuls after the gather lands

_CACHE = {}


def _build(attn_scale: float, res_scale: float):
    nc = bacc.Bacc("TRN2", target_bir_lowering=False, debug=False,
                   enable_asserts=False, num_devices=NCORES)

    xT_d = nc.dram_tensor("xT", [DIM, NLOC], BF16, kind="ExternalInput").ap()
    AT_d = nc.dram_tensor("AT", [DIM, INNER], BF16, kind="ExternalInput").ap()
    fxdt = FP8 if FP8_FX else BF16
    WfxT_d = nc.dram_tensor("WfxT", [DIM, INNER], fxdt, kind="ExternalInput").ap()
    if FP8_FX:
        xT8_d = nc.dram_tensor("xT8", [DIM, NLOC], FP8, kind="ExternalInput").ap()
    idbf_d = nc.dram_tensor("idbf", [128, 128], BF16, kind="ExternalInput").ap()
    id32_d = nc.dram_tensor("id32", [64, 64], F32, kind="ExternalInput").ap()
    WqT_d = nc.dram_tensor("WqT", [2 * D, D], F32, kind="ExternalInput").ap()
    WkT_d = nc.dram_tensor("WkT", [D, D], F32, kind="ExternalInput").ap()
    WvT_d = nc.dram_tensor("WvT", [D, D], F32, kind="ExternalInput").ap()
    WoT_d = nc.dram_tensor("WoT", [128, H, DIM], BF16, kind="ExternalInput").ap()
    odt = BF16 if BF16_OUT else F32
    outT_d = nc.dram_tensor("outT", [2, 128, NLOC], odt, kind="ExternalOutput").ap()
    warm_d = nc.dram_tensor("warm", [64, 64], F32, kind="ExternalOutput").ap()

    xT_v = xT_d.rearrange("(c p) n -> p c n", p=128)    # [128, 2, NLOC]
    AT_v = AT_d.rearrange("(c p) n -> p c n", p=128)    # [128, 2, 512]
    WfxT_v = WfxT_d.rearrange("(c p) n -> p c n", p=128)
    if FP8_FX:
        xT8_v = xT8_d.rearrange("(c p) n -> p c n", p=128)

    ccdt = BF16 if BF16_CC else F32
    E = D + 1

    with tile.TileContext(nc) as tc:
        with (
            tc.tile_pool(name="consts", bufs=1) as consts,
            tc.tile_pool(name="store", bufs=1) as store,
            tc.tile_pool(name="work", bufs=PIPE_D + 2) as work,
            tc.tile_pool(name="xtp", bufs=PF + 2) as xtp,
            tc.tile_pool(name="swp", bufs=8) as swp,
            tc.tile_pool(name="small", bufs=PIPE_D + 2) as small,
            tc.tile_pool(name="obuf", bufs=2) as obuf,
            tc.tile_pool(name="stage", bufs=1) as stg_pool,
            tc.tile_pool(name="pslg", bufs=2, space="PSUM") as pslg,
            tc.tile_pool(name="psfx", bufs=2, space="PSUM") as psfx,
            tc.tile_pool(name="psacc", bufs=1, space="PSUM") as psacc,
            tc.tile_pool(name="psst", bufs=2, space="PSUM") as psst,
            tc.tile_pool(name="dram", bufs=1, space="DRAM") as dram,
        ):
            # ---- resident constants (split across the two DGE queues) ----
            AT_sb = consts.tile([128, KCH, INNER], BF16)
            nc.sync.dma_start(AT_sb, AT_v)
            WfxT_sb = consts.tile([128, KCH, INNER], fxdt)
            nc.scalar.dma_start(WfxT_sb, WfxT_v)
            id32 = consts.tile([64, 64], F32)
            nc.sync.dma_start(id32, id32_d)
            idbf = None
            if not XBAR_T:
                idbf = consts.tile([128, 128], BF16)
                nc.scalar.dma_start(idbf, idbf_d)
            WqT2_sb = consts.tile([128, 64], F32)
            nc.sync.dma_start(WqT2_sb, WqT_d)
            WkT_sb = consts.tile([64, 64], F32)
            nc.scalar.dma_start(WkT_sb, WkT_d)
            WvT_sb = consts.tile([64, 64], F32)
            nc.sync.dma_start(WvT_sb, WvT_d)
            WoT2_sb = consts.tile([128, H, DIM], BF16)
            nc.scalar.dma_start(WoT2_sb, WoT_d)

            # transposed routing weights, tile-major: [hg%128, t, chunk, tok]
            swT_store = store.tile([128, T, 4, TOK], BF16)
            # slice-token accumulator: [p, hp, blk, 0:64]=st_un for head
            # h=2*blk+hp at partitions hp*64+g, col 64 = snorm. dim1 selects
            # a bank per partition-half so each half owns its zero region.
            st_ps = psacc.tile([128, 2, 4, 128], F32, name="st_ps")

            # exchange buffers (DRAM round-trip, proven HW path)
            cc_in = [dram.tile([128, 4 * E], ccdt, name=f"cc_in{i}")
                     for i in range(2)]
            cc_out = [dram.tile([2, 128, 4 * E], ccdt, name=f"cc_out{i}")
                      for i in range(2)]

            # preset the ones column of the rotating fxs buffers
            fxs_bufs = []
            for i in range(PIPE_D + 2):
                fxs = work.tile([128, H, E], BF16, tag="fxs")
                nc.gpsimd.memset(fxs[:, :, D], 1.0)
                fxs_bufs.append(fxs)

            # ================= PASS 1 =================
            # software-pipelined: iteration t runs main/softmax for tile t
            # and pooling/transpose for tile t - PIPE_D, so the PE never
            # waits on the softmax chain. xt loads, swn buffers and XBAR
            # transposes are paired over two tiles to halve DGE-queue
            # instruction overhead.
            swn_hist = {}
            fxs_hist = {}
            xt_hist = {}
            stun = [None, None]

            def issue_xt(p):
                if p >= T // 2:
                    return
                t0 = 2 * p
                xt2 = xtp.tile([128, KCH, 2 * TOK], BF16, tag="xt",
                               name=f"xt{t0}")
                nc.sync.dma_start(
                    xt2, xT_v[:, :, t0 * TOK:(t0 + 2) * TOK])
                xt_hist[t0] = xt_hist[t0 + 1] = xt2
                if FP8_FX:
                    xt28 = xtp.tile([128, KCH, 2 * TOK], FP8, tag="xt8",
                                    name=f"xt8_{t0}")
                    nc.sync.dma_start(
                        xt28, xT8_v[:, :, t0 * TOK:(t0 + 2) * TOK])
                    xt_hist[(t0, 8)] = xt_hist[(t0 + 1, 8)] = xt28

            def head_block(t):
                if t % 2 == 0:
                    issue_xt(t // 2 + PF)
                o = (t % 2) * TOK
                xt = xt_hist.pop(t)
                lg = pslg.tile([128, INNER], F32, tag="lg")
                for k in range(KCH):
                    nc.tensor.matmul(lg, xt[:, k, o:o + TOK], AT_sb[:, k, :],
                                     start=(k == 0), stop=(k == KCH - 1))
                fx = psfx.tile([128, INNER], F32, tag="fx")
                if FP8_FX:
                    xt8 = xt_hist.pop((t, 8))
                    nc.tensor.matmul(fx, xt8[:, :, o:o + TOK], WfxT_sb,
                                     perf_mode=DR, start=True, stop=True)
                else:
                    for k in range(KCH):
                        nc.tensor.matmul(fx, xt[:, k, o:o + TOK],
                                         WfxT_sb[:, k, :],
                                         start=(k == 0), stop=(k == KCH - 1))

                usw = work.tile([128, H, G], BF16, tag="usw")
                nc.scalar.activation(usw.rearrange("p h g -> p (h g)"),
                                     lg, AF.Exp)
                den = small.tile([128, H], F32, tag="den")
                nc.vector.reduce_sum(den, usw, axis=mybir.AxisListType.X)
                rden = small.tile([128, H], F32, tag="rden")
                nc.vector.reciprocal(rden, den)
                if t % 2 == 0:
                    swn_hist[t // 2] = swp.tile([128, 2, INNER], BF16,
                                                tag="swnp",
                                                name=f"swnp{t // 2}")
                swnp = swn_hist[t // 2]
                nc.gpsimd.tensor_tensor(
                    swnp[:, t % 2].rearrange("p (h g) -> p h g", g=G), usw,
                    rden[:, :, None].to_broadcast([128, H, G]),
                    ALU.mult)
                fxs = fxs_bufs[t % (PIPE_D + 2)]
                if t % 2 == 0:
                    nc.scalar.copy(
                        fxs[:, :, 0:D], fx.rearrange("p (h d) -> p h d", d=D))
                else:
                    nc.vector.tensor_copy(
                        fxs[:, :, 0:D], fx.rearrange("p (h d) -> p h d", d=D))
                fxs_hist[t] = fxs

            def tail_block(t):
                swnp = swn_hist[t // 2]
                swn2 = swnp[:, t % 2]
                fxs = fxs_hist.pop(t)
                first, last = t % 64 == 0, t % 64 == 63
                for blk in range(4):
                    for hp in range(2):
                        lhs = swn2[:, blk * 128 + hp * 64:
                                   blk * 128 + (hp + 1) * 64]
                        nc.tensor.matmul(
                            st_ps[64 * hp:64 * (hp + 1), hp, blk, 0:E],
                            lhs, fxs[:, 2 * blk + hp, :],
                            start=(first and blk == 0),
                            stop=(last and blk == 3))
                if last:
                    # emit the exchange ahead of this pair's transpose so the
                    # cc_in DMA isn't queued behind it on the sync DGE
                    half = t // 64
                    stun[half] = stg_pool.tile([128, 4, E], ccdt,
                                               name=f"stun{half}")
                    nc.vector.tensor_copy(stun[half][0:64],
                                          st_ps[0:64, 0, :, 0:E])
                    nc.scalar.copy(stun[half][64:128],
                                   st_ps[64:128, 1, :, 0:E])
                    nc.sync.dma_start(
                        cc_in[half],
                        stun[half].rearrange("p a b -> p (a b)"))
                    nc.gpsimd.collective_compute(
                        "AllGather", ALU.bypass,
                        replica_groups=[[0, 1], [2, 3], [4, 5], [6, 7]],
                        ins=[cc_in[half].opt()],
                        outs=[cc_out[half].rearrange(
                            "r p e -> (r p) e").opt()],
                    )
                if XBAR_T:
                    if t % 2 == 1:
                        # one XBAR transpose covers both tiles of the pair:
                        # in [128, 1024] -> out [128, 8, 128] which is
                        # exactly swT_store[:, t-1:t+1] flattened.
                        nc.sync.dma_start_transpose(
                            swT_store[:, t - 1:t + 1].rearrange(
                                "p a b c -> p (a b) c"),
                            swnp.rearrange("p a b -> p (a b)"))
                        del swn_hist[t // 2]
                else:
                    swtp = psst.tile([128, 4, TOK], BF16, tag="tp")
                    for blk in range(4):
                        nc.tensor.transpose(swtp[:, blk, :],
                                            swn2[:, ts(blk, 128)], idbf)
                    nc.vector.tensor_copy(swT_store[:, t], swtp)
                    if t % 2 == 1:
                        del swn_hist[t // 2]

            for p in range(PF):
                issue_xt(p)
            for t in range(T + PIPE_D):
                if t < T:
                    head_block(t)
                if t >= PIPE_D:
                    tail_block(t - PIPE_D)

            # ============ STAGE (slice attention, tiny) ============
            # load reduced halves back with g on partitions:
            # cc_out[i] flat = [hp*64+g, blk, E] -> [g, r, hp, blk, E]
            gth = [None, None]
            for i in range(2):
                gth[i] = stg_pool.tile([64, 2, 2, 4, E], ccdt,
                                       name=f"gth{i}")
                src = cc_out[i].rearrange("r (hp g) (blk e) -> g r hp blk e",
                                          hp=2, e=E)
                (nc.sync if i == 0 else nc.scalar).dma_start(gth[i], src)

            # PE warm-up: junk accumulating matmuls reading the gathered tile
            # keep HAM from idling through the vector prologue below. The
            # accumulator is sunk to a throwaway output to satisfy the BIR
            # verifier (every written location needs a reader).
            gflat = gth[1].rearrange("g r hp blk e -> g (r hp blk e)")
            wtp = psst.tile([64, 64], F32, tag="tp", name="wtp_acc")
            for w in range(WARM_MM):
                sl = (w % 15) * 64
                nc.tensor.matmul(wtp, gflat[:, sl:sl + 64],
                                 gflat[:, sl + 64:sl + 128],
                                 start=(w == 0), stop=(w == WARM_MM - 1))
            warm_sb = stg_pool.tile([64, 64], F32, name="warm_sb")
            nc.vector.tensor_copy(warm_sb, wtp)
            nc.sync.dma_start(warm_d, warm_sb)

            # stg[g, hp, blk, e], heads h = 2*blk + hp
            stg = stg_pool.tile([64, 2, 4, E], F32)
            s0 = stg_pool.tile([64, 2, 4, E], F32, name="s0")
            nc.vector.tensor_add(s0, gth[0][:, 0], gth[0][:, 1])
            nc.gpsimd.tensor_add(stg, s0, gth[1][:, 0])
            nc.vector.tensor_add(stg, stg, gth[1][:, 1])
            stgh = stg.rearrange("g hp blk e -> g (hp blk) e")  # kh = hp*4+blk

            snorm_e = stg_pool.tile([64, H], F32)
            nc.vector.tensor_scalar_add(snorm_e, stgh[:, :, D], EPS_SLICE)
            rs = stg_pool.tile([64, H], F32)
            nc.vector.reciprocal(rs, snorm_e)
            st_sb = stg_pool.tile([64, H, D], F32)
            nc.vector.tensor_tensor(st_sb, stgh[:, :, 0:D],
                                    rs[:, :, None].to_broadcast([64, H, D]),
                                    ALU.mult)
            kv = stg_pool.tile([64, D], F32)
            nc.vector.reduce_sum(kv, st_sb.rearrange("p h d -> p d h"),
                                 axis=mybir.AxisListType.X)

            stT = stg_pool.tile([64, H, D], F32)
            for kh in range(H):
                tp = psst.tile([64, 64], F32, tag="tp")
                nc.tensor.transpose(tp, st_sb[:, kh, :], id32)
                nc.vector.tensor_copy(stT[:, kh, :], tp)
            kvT_p = psst.tile([64, 64], F32, tag="tp")
            nc.tensor.transpose(kvT_p, kv, id32)
            kvT = stg_pool.tile([64, D], F32)
            nc.vector.tensor_copy(kvT, kvT_p)

            q_ps = pslg.tile([64, H, D], F32, tag="lg", name="q_ps")
            for kh in range(H):
                nc.tensor.matmul(q_ps[:, kh, :], stT[:, kh, :], WqT2_sb[0:64],
                                 start=(kh == 0), stop=(kh == H - 1))
            k_ps = psfx.tile([64, D], F32, tag="fx", name="k_ps")
            nc.tensor.matmul(k_ps, kvT, WkT_sb, start=True, stop=True)
            k_sb = stg_pool.tile([64, D], F32)
            nc.vector.tensor_copy(k_sb, k_ps)
            v_ps = psfx.tile([64, D], F32, tag="fx", name="v_ps")
            nc.tensor.matmul(v_ps, kvT, WvT_sb, start=True, stop=True)
            v_sb = stg_pool.tile([64, D], F32)
            nc.vector.tensor_copy(v_sb, v_ps)

            def rnorm(src, nh, tag):
                # 1/||src|| = sqrt(1/sum(src^2)) without Ln/Exp table loads
                sq = stg_pool.tile([64, nh, D], F32, name=f"sq_{tag}")
                nc.scalar.activation(sq, src, AF.Square)
                n2 = stg_pool.tile([64, nh], F32, name=f"n2_{tag}")
                nc.vector.reduce_sum(n2, sq, axis=mybir.AxisListType.X)
                rn2 = stg_pool.tile([64, nh], F32, name=f"rn2_{tag}")
                nc.vector.reciprocal(rn2, n2)
                t1 = stg_pool.tile([64, nh], F32, name=f"t1_{tag}")
                nc.scalar.sqrt(t1, rn2)
                return t1

            rq = rnorm(q_ps, H, "q")
            rk = rnorm(k_sb[:, None, :], 1, "k")

            qn = stg_pool.tile([64, H, D], F32)
            nc.vector.tensor_tensor(qn, q_ps,
                                    rq[:, :, None].to_broadcast([64, H, D]),
                                    ALU.mult)
            kn = stg_pool.tile([64, D], F32)
            nc.vector.tensor_tensor(kn, k_sb,
                                    rk[:, 0:1].to_broadcast([64, D]), ALU.mult)

            qnT = stg_pool.tile([64, H, D], F32)
            for kh in range(H):
                tp = psst.tile([64, 64], F32, tag="tp")
                nc.tensor.transpose(tp, qn[:, kh, :], id32)
                nc.vector.tensor_copy(qnT[:, kh, :], tp)
            knT_p = psst.tile([64, 64], F32, tag="tp")
            nc.tensor.transpose(knT_p, kn, id32)
            knT = stg_pool.tile([64, D], F32)
            nc.vector.tensor_copy(knT, knT_p)

            L_ps = pslg.tile([64, H, G], F32, tag="lg", name="L_ps")
            for kh in range(H):
                nc.tensor.matmul(L_ps[:, kh, :], qnT[:, kh, :], knT,
                                 start=(kh == 0), stop=(kh == H - 1))
            e_sb = stg_pool.tile([64, H, G], F32)
            nc.scalar.activation(e_sb, L_ps, AF.Exp, scale=attn_scale)
            aden = stg_pool.tile([64, H], F32)
            nc.vector.reduce_sum(aden, e_sb, axis=mybir.AxisListType.X)
            ra = stg_pool.tile([64, H], F32)
            nc.vector.reciprocal(ra, aden)

            LT_ps = psfx.tile([64, H, G], F32, tag="fx", name="LT_ps")
            for kh in range(H):
                nc.tensor.matmul(LT_ps[:, kh, :], knT, qnT[:, kh, :],
                                 start=(kh == 0), stop=(kh == H - 1))
            eT_sb = stg_pool.tile([64, H, G], F32)
            nc.scalar.activation(eT_sb, LT_ps, AF.Exp, scale=attn_scale)

            av_ps = pslg.tile([64, H, D], F32, tag="lg", name="av_ps")
            for kh in range(H):
                nc.tensor.matmul(av_ps[:, kh, :], eT_sb[:, kh, :], v_sb,
                                 start=(kh == 0), stop=(kh == H - 1))

            os_sb = stg_pool.tile([64, H, D], F32)
            nc.vector.tensor_tensor(os_sb, av_ps,
                                    ra[:, :, None].to_broadcast([64, H, D]),
                                    ALU.mult)
            rst = stg_pool.tile([64, H, D], F32)
            nc.vector.tensor_scalar_mul(rst, st_sb, res_scale)
            nc.vector.tensor_add(os_sb, os_sb, rst)

            osT = stg_pool.tile([64, H, D], BF16)
            for kh in range(H):
                tp = psst.tile([64, 64], F32, tag="tp")
                nc.tensor.transpose(tp, os_sb[:, kh, :], id32)
                nc.vector.tensor_copy(osT[:, kh, :], tp)

            C_sb = stg_pool.tile([128, 4, DIM], BF16)
            for cc in range(4):
                C_ps = psst.tile([128, DIM], F32, tag="tp")
                for par in range(2):
                    kh = par * 4 + cc
                    h = 2 * cc + par
                    nc.tensor.matmul(C_ps[64 * par:64 * par + 64, :],
                                     osT[:, kh, :], WoT2_sb[0:64, h, :],
                                     start=True, stop=True)
                nc.vector.tensor_copy(C_sb[:, cc, :], C_ps)

            # ================= PASS 2 =================
            # outT[f, tok] accumulated over 4 hg-chunks; C slices stationary.
            # op buffers rotate over 4 PSUM banks (pslg + psfx pools).
            QT = 4                       # token-tiles per group
            GRP = QT * TOK               # 512
            NG = NLOC // GRP             # 32
            ob = None
            for fb in range(2):
                for g in range(NG):
                    pool = (pslg, psfx)[g % 2]
                    op = pool.tile([128, QT, TOK], F32,
                                   tag=("lg", "fx")[g % 2], name="op")
                    for cc in range(4):
                        nc.tensor.matmul(
                            op,
                            C_sb[:, cc, fb * 128:(fb + 1) * 128],
                            swT_store[:, g * QT:(g + 1) * QT, cc, :],
                            start=(cc == 0), stop=(cc == 3))
                    if g % 2 == 0:
                        ob = obuf.tile([128, 2, GRP], odt, tag="ob")
                        nc.vector.tensor_copy(ob[:, 0, :],
                                              op.rearrange("p a b -> p (a b)"))
                    else:
                        nc.scalar.copy(ob[:, 1, :],
                                       op.rearrange("p a b -> p (a b)"))
                        eng = nc.sync if (g // 2) % 2 == 0 else nc.scalar
                        eng.dma_start(
                            outT_d[fb, :, (g - 1) * GRP:(g + 1) * GRP],
                            ob.rearrange("p a b -> p (a b)"))

    nc.finalize()
    return nc


def prepare(x, Wfx, bfx, Wx, bx, Wslice, bslice, temp, Wq, Wk, Wv,
            res_scale, attn_scale, Wout, bout):
    x = np.asarray(x, dtype=np.float32)
    Wfx = np.asarray(Wfx, np.float32); bfx = np.asarray(bfx, np.float32)
    Wx = np.asarray(Wx, np.float32); bx = np.asarray(bx, np.float32)
    Wslice = np.asarray(Wslice, np.float32); bslice = np.asarray(bslice, np.float32)
    temp = np.asarray(temp, np.float32).reshape(H)
    Wq = np.asarray(Wq, np.float32); Wk = np.asarray(Wk, np.float32)
    Wv = np.asarray(Wv, np.float32)
    res_scale_f = float(np.asarray(res_scale, np.float32))
    attn = np.asarray(attn_scale, np.float32).reshape(H)
    Wout = np.asarray(Wout, np.float32); bout = np.asarray(bout, np.float32)

    assert np.all(np.abs(bfx) == 0) and np.all(np.abs(bx) == 0) \
        and np.all(np.abs(bslice) == 0), "nonzero projection biases unsupported"
    assert np.ptp(attn) == 0, "non-uniform attn_scale unsupported"
    attn_f = float(attn[0])

    # folded logits weight: logits[:, h*G+g] = x @ ((Wslice @ Wx_h)/temp_h).T
    A = np.concatenate(
        [(Wslice @ Wx[h * D:(h + 1) * D, :]) / temp[h] for h in range(H)], axis=0)
    AT = np.ascontiguousarray(A.T).astype(ml_dtypes.bfloat16)
    if FP8_FX:
        WfxT = np.ascontiguousarray(Wfx.T).astype(NP_FP8)
    else:
        WfxT = np.ascontiguousarray(Wfx.T).astype(ml_dtypes.bfloat16)
    WoT1 = Wout.T.reshape(H, D, DIM).transpose(1, 0, 2)                # [64, 8, 256]
    WoT = np.ascontiguousarray(
        np.concatenate([WoT1, WoT1], axis=0)).astype(ml_dtypes.bfloat16)
    WqT1 = np.ascontiguousarray(Wq.T)
    WqT = np.concatenate([WqT1, WqT1], axis=0)                         # [128, 64]
    WkT = np.ascontiguousarray(Wk.T) / H
    WvT = np.ascontiguousarray(Wv.T) / H
    idbf = np.eye(128, dtype=np.float32).astype(ml_dtypes.bfloat16)
    id32 = np.eye(64, dtype=np.float32)

    key = (attn_f, res_scale_f)
    if key not in _CACHE:
        _CACHE[key] = _build(attn_f, res_scale_f)
    nc = _CACHE[key]

    in_maps = []
    for c in range(NCORES):
        b, half = c // 2, c % 2
        xs = x[b, half * NLOC:(half + 1) * NLOC, :]       # [16384, 256]
        xT = np.ascontiguousarray(xs.T.astype(ml_dtypes.bfloat16))
        im = dict(xT=xT, AT=AT, WfxT=WfxT, idbf=idbf, id32=id32,
                  WqT=WqT, WkT=WkT, WvT=WvT, WoT=WoT)
        if FP8_FX:
            im["xT8"] = np.ascontiguousarray(xs.T.astype(NP_FP8))
        in_maps.append(im)

    def gather(core_outs):
        out = np.empty((B, N, DIM), np.float32)
        for c in range(NCORES):
            b, half = c // 2, c % 2
            oT = np.asarray(core_outs[c]).reshape(DIM, NLOC)
            out[b, half * NLOC:(half + 1) * NLOC, :] = \
                oT.T.astype(np.float32)
        if np.any(bout):
            out += bout
        return out

    return dict(nc=nc, in_maps=in_maps, gather=gather)


def kernel(**inputs):
    prep = prepare(**inputs)
    global _LAST_IN_MAPS
    _LAST_IN_MAPS = prep["in_maps"]
    res = bass_utils.run_bass_kernel_spmd(
        prep["nc"], prep["in_maps"], core_ids=list(range(NCORES)))
    return prep["gather"]([res.results[c]["outT"] for c in range(NCORES)])
